# revision 1
# baseline (speedup 1.0000x reference)
"""BEiT-style attention (B=128, N=197, C=768, H=12) on 8 TRN2 NeuronCores.

Strategy: pure data parallelism over batch — each core processes 16
samples end-to-end; no collectives. Host pre-gathers the per-sample
bitfit biases (b_idx lookups), pre-transposes x to [C, N] per sample,
folds the attention scale into the q weights/bias, and pre-computes
exp(rel_pos_bias)^T so softmax(S + rpb) = normalize(exp(S) * exp_rpbT).

Device per sample:
  qkT  [1536,197] = w_qk @ x^T          (weights stationary, 2-sample batch)
  v    [197, 768] = x @ w_v^T + v_bias  (x^T stationary slices)
  per head: S^T[m,n] = k_h @ q_h^T  (two heads packed in the 128-row PE
  array via tile_position), P = exp(S^T) * exp_rpbT, out^T[hd+1, n] =
  [v_h | 1]^T @ P  (ones column yields the softmax denominator),
  normalize with a reciprocal broadcast via a ones-matmul,
  y^T [768,197] = w_proj @ out_allT + b_proj.
Host transposes the gathered y^T back to [B, N, C].
"""
import numpy as np
import ml_dtypes

import concourse.bass as bass
import concourse.tile as tile
from concourse import mybir
from concourse.bass_utils import run_bass_kernel_spmd

B, N, C = 128, 197, 768
H, HD = 12, 64
NCORES = 8
BL = B // NCORES          # 16 samples per core
NPAIRS = BL // 2          # 8 sample pairs
N2 = 2 * N                # 394
KT = C // 128             # 6 k-tiles
MT_QK = 2 * C // 128      # 12 m-tiles of qkT
BF16 = mybir.dt.bfloat16
F32 = mybir.dt.float32
AF = mybir.ActivationFunctionType
PV_TRAIL = 1


def _split_sync_waits(nc, max_waits=1, max_updates=1):
    """TPB descriptors have ONE wait and ONE update slot; hoist extras
    onto same-engine NoOps (trailing-nop updates are completion-safe)."""
    n_split = 0
    for f in nc.m.functions:
        for bb in f.blocks:
            old = list(bb.instructions)
            new = []
            changed = False
            for ins in old:
                si = ins.sync_info
                tname = type(ins).__name__
                is_dma = ("TensorLoad" in tname or "TensorSave" in tname
                          or "TensorCopy" in tname or "TriggeredCopy" in tname)
                if si is None or is_dma:
                    new.append(ins)
                    continue
                waits = list(si.on_wait)
                updates = list(si.on_update)
                if len(waits) <= max_waits and len(updates) <= max_updates:
                    new.append(ins)
                    continue
                changed = True
                n_split += 1
                while len(waits) > max_waits:
                    w = waits.pop(0)
                    new.append(mybir.InstNoOp(
                        name=nc.get_next_instruction_name(), engine=ins.engine,
                        sync_info=mybir.SyncInfo(on_wait=[w], on_update=[]),
                        bass_nofuse=True))
                post = []
                while len(updates) > max_updates:
                    u = updates.pop()
                    post.append(mybir.InstNoOp(
                        name=nc.get_next_instruction_name(), engine=ins.engine,
                        sync_info=mybir.SyncInfo(on_wait=[], on_update=[u]),
                        bass_nofuse=True))
                ins.sync_info = mybir.SyncInfo(on_wait=waits, on_update=updates)
                new.append(ins)
                new.extend(post)
            if changed:
                bb.instructions = new
    return n_split


def build_nc(repeat=1):
    nc = bass.Bass("TRN2")
    xt_d = nc.declare_dram_parameter("xt", [BL, 128, KT * N], BF16, isOutput=False)
    wqkv_d = nc.declare_dram_parameter("wqkv", [128, KT, 3 * C], BF16, isOutput=False)
    wproj_d = nc.declare_dram_parameter("wproj", [128, KT, C], BF16, isOutput=False)
    rpb_d = nc.declare_dram_parameter("rpb", [N, H, N], BF16, isOutput=False)
    qkvb_d = nc.declare_dram_parameter("qkvb", [128, MT_QK, BL], F32, isOutput=False)
    projb_d = nc.declare_dram_parameter("projb", [128, KT, BL], F32, isOutput=False)
    vb_d = nc.declare_dram_parameter("vb", [BL, H * HD], BF16, isOutput=False)
    out_d = nc.declare_dram_parameter("out", [BL, C, N], F32, isOutput=True)

    with tile.TileContext(nc) as tc:
        with (
            tc.tile_pool(name="const", bufs=1) as const,
            tc.tile_pool(name="xtp", bufs=3) as xtp,
            tc.tile_pool(name="qkp", bufs=3) as qkp,
            tc.tile_pool(name="vp", bufs=4) as vp,
            tc.tile_pool(name="vbp", bufs=4) as vbp,
            tc.tile_pool(name="esp", bufs=8) as esp,
            tc.tile_pool(name="rcp", bufs=7) as rcp,
            tc.tile_pool(name="oap", bufs=3) as oap,
            tc.tile_pool(name="yp", bufs=3) as yp,
            tc.tile_pool(name="psA", bufs=2, space="PSUM") as psA,
            tc.tile_pool(name="psS", bufs=2, space="PSUM") as psS,
            tc.tile_pool(name="psB", bufs=2, space="PSUM") as psB,
        ):
            # ---- resident constants ----
            wqkv_sb = const.tile([128, KT, 3 * C], BF16)
            nc.sync.dma_start(wqkv_sb, wqkv_d[:])
            wproj_sb = const.tile([128, KT, C], BF16)
            nc.sync.dma_start(wproj_sb, wproj_d[:])
            rpb0 = const.tile([128, H, N], BF16)
            nc.sync.dma_start(rpb0, rpb_d[0:128])
            rpb1 = const.tile([69, H, N], BF16)
            nc.sync.dma_start(rpb1, rpb_d[128:N])
            ones64 = const.tile([1, 64], BF16)
            nc.vector.memset(ones64, 1.0)
            qkvb_sb = const.tile([128, MT_QK, BL], F32)
            nc.sync.dma_start(qkvb_sb, qkvb_d[:])
            projb_sb = const.tile([128, KT, BL], F32)
            nc.sync.dma_start(projb_sb, projb_d[:])

            for _rep in range(repeat):
                st = {}

                def emit_load(p):
                    sg = (2 * p, 2 * p + 1)
                    xt = xtp.tile([128, KT, N2], BF16, name="xt")
                    for s in range(2):
                        nc.sync.dma_start(
                            xt[:, :, s * N:(s + 1) * N],
                            xt_d[sg[s]].rearrange("p (k n) -> p k n", k=KT))
                    vb_bc = [None, None]
                    for s in range(2):
                        vb_bc[s] = vbp.tile([128, H, HD], BF16, tag="vb", name="vb")
                        nc.scalar.dma_start(
                            vb_bc[s],
                            vb_d[sg[s]:sg[s] + 1, :].rearrange(
                                "o (h d) -> o h d", h=H).to_broadcast([128, H, HD]))
                    qkT = qkp.tile([128, MT_QK, N2], BF16, name="qkT")
                    st[p] = {"xt": xt, "vb": vb_bc, "qkT": qkT, "v": None}

                def emit_qkv_m(p, m):
                    sg = (2 * p, 2 * p + 1)
                    xt, qkT = st[p]["xt"], st[p]["qkT"]
                    ps = psA.tile([128, N2], F32, tag="mm", name="ps")
                    for k in range(KT):
                        nc.tensor.matmul(ps, wqkv_sb[:, k, m * 128:(m + 1) * 128],
                                         xt[:, k, :], start=(k == 0),
                                         stop=(k == KT - 1))
                    for s in range(2):
                        dst = qkT[:, m, s * N:(s + 1) * N]
                        src = ps[:, s * N:(s + 1) * N]
                        bias = qkvb_sb[:, m, sg[s]:sg[s] + 1]
                        if m % 2 == 0:
                            nc.scalar.activation(dst, src, AF.Identity,
                                                 bias=bias, scale=1.0)
                        else:
                            nc.vector.tensor_scalar_add(dst, src, bias)

                def emit_v_chunk(p, s, nt):
                    xt, vb_bc = st[p]["xt"], st[p]["vb"]
                    if st[p]["v"] is None:
                        st[p]["v"] = [[None, None], [None, None]]
                    nts = 128 if nt == 0 else N - 128
                    vt = vp.tile([nts, H, HD + 1], BF16, tag=f"v{nt}", name="vt")
                    nc.vector.memset(vt[:, :, HD:HD + 1], 1.0)
                    for half in range(2):
                        ps = psA.tile([128, N2], F32, tag="mm", name="ps")
                        for k in range(KT):
                            nc.tensor.matmul(
                                ps[:nts, 0:384],
                                xt[:, k, s * N + nt * 128:
                                   s * N + nt * 128 + nts],
                                wqkv_sb[:, k, 2 * C + half * 384:
                                        2 * C + (half + 1) * 384],
                                start=(k == 0), stop=(k == KT - 1))
                        nc.scalar.activation(
                            vt[:, half * 6:(half + 1) * 6, 0:HD],
                            ps[:nts, 0:384].rearrange("p (h d) -> p h d", h=6),
                            AF.Copy)
                    nc.vector.tensor_add(vt[:, :, 0:HD], vt[:, :, 0:HD],
                                         vb_bc[s][:nts])
                    st[p]["v"][s][nt] = vt

                def emit_v(p):
                    for s in range(2):
                        for nt in range(2):
                            emit_v_chunk(p, s, nt)

                def emit_s(p, s, hp):
                    # The two concurrent row-packed matmuls drain into the
                    # two DIFFERENT banks of one 2-bank psum tile (same-bank
                    # concurrent drain is a HW error), so exp and the rpb
                    # multiply each run as ONE wide instruction.
                    qkT = st[p]["qkT"]
                    ha = 2 * hp
                    es = [None, None]
                    for mt in range(2):
                        mts = 128 if mt == 0 else N - 128
                        rpb = rpb0 if mt == 0 else rpb1
                        pss = psS.tile([128, 2, 512], F32, tag="s2", name="pss")
                        for hh, (pl, pr) in enumerate(((0, 64), (64, 128))):
                            nc.tensor.matmul(
                                pss[:mts, hh, 0:N],
                                qkT[pl:pr, KT + hp,
                                    s * N + mt * 128: s * N + mt * 128 + mts],
                                qkT[pl:pr, hp, s * N:(s + 1) * N],
                                start=True, stop=True, tile_position=(pl, 0))
                        e = esp.tile([mts, N2], BF16, tag=f"es{mt}", name="es")
                        nc.scalar.activation(
                            e.rearrange("p (a n) -> p a n", a=2),
                            pss[:mts, :, 0:N], AF.Exp)
                        nc.vector.tensor_mul(
                            e, e,
                            rpb[:, ha:ha + 2, :].rearrange("p a n -> p (a n)"))
                        es[mt] = e
                    return es

                def emit_pv(p, s, hp, es, oa):
                    v_sb = st[p]["v"]
                    pvt = psB.tile([HD + 1, N2], F32, tag="pv", name="pvt")
                    for mt in range(2):
                        for hh, h in enumerate((2 * hp, 2 * hp + 1)):
                            # start=True clears has_written for the WHOLE bank:
                            # only the very first matmul may set it.
                            nc.tensor.matmul(
                                pvt[:, hh * N:(hh + 1) * N],
                                v_sb[s][mt][:, h, :],
                                es[mt][:, hh * N:(hh + 1) * N],
                                start=(mt == 0 and hh == 0),
                                stop=(mt == 1 and hh == 1))
                    ssum = rcp.tile([1, N2], BF16, tag="rc", name="ssum")
                    nc.scalar.copy(ssum, pvt[HD:HD + 1, :])
                    bc = psB.tile([64, N2], F32, tag="pv", name="bc")
                    nc.tensor.matmul(bc, ones64, ssum, start=True, stop=True)
                    bc_sb = rcp.tile([64, N2], BF16, tag="bc_sb", name="bc_sb")
                    with nc.allow_low_precision(reason="softmax denom in bf16"):
                        nc.vector.reciprocal(bc_sb, bc)
                    for hh in range(2):
                        nc.vector.tensor_mul(
                            oa[hh * 64:(hh + 1) * 64, hp, s * N:(s + 1) * N],
                            pvt[0:HD, hh * N:(hh + 1) * N],
                            bc_sb[:, hh * N:(hh + 1) * N])

                def emit_proj(p):
                    sg = (2 * p, 2 * p + 1)
                    oa = st[p]["oa"]
                    for m in range(KT):
                        ps = psA.tile([128, N2], F32, tag="mm", name="ps")
                        for k in range(KT):
                            nc.tensor.matmul(
                                ps, wproj_sb[:, k, m * 128:(m + 1) * 128],
                                oa[:, k, :], start=(k == 0), stop=(k == KT - 1))
                        y = yp.tile([128, N2], F32, tag="y", name="y")
                        for s in range(2):
                            dst = y[:, s * N:(s + 1) * N]
                            src = ps[:, s * N:(s + 1) * N]
                            bias = projb_sb[:, m, sg[s]:sg[s] + 1]
                            if m % 2 == 0:
                                nc.scalar.activation(dst, src, AF.Identity,
                                                     bias=bias, scale=1.0)
                            else:
                                nc.vector.tensor_scalar_add(dst, src, bias)
                        for s in range(2):
                            nc.sync.dma_start(
                                out_d[sg[s], m * 128:(m + 1) * 128, :],
                                y[:, s * N:(s + 1) * N])

                # skewed pipeline: pair p attention carries pair p+1 QKV
                emit_load(0)
                for m in range(MT_QK):
                    emit_qkv_m(0, m)
                emit_v(0)
                for p in range(NPAIRS):
                    if p + 1 < NPAIRS:
                        emit_load(p + 1)
                    oa = oap.tile([128, KT, N2], BF16, name="oa")
                    st[p]["oa"] = oa
                    pending = []
                    iters = [(s, hp) for s in range(2) for hp in range(H // 2)]
                    for i, (s, hp) in enumerate(iters):
                        es = emit_s(p, s, hp)
                        if p + 1 < NPAIRS:
                            if i < 8:
                                emit_qkv_m(p + 1, i)
                            else:
                                emit_v_chunk(p + 1, (i - 8) // 2, (i - 8) % 2)
                        pending.append((s, hp, es))
                        if len(pending) > PV_TRAIL:
                            emit_pv(p, *pending.pop(0), oa)
                    for item in pending:
                        emit_pv(p, *item, oa)
                    if p + 1 < NPAIRS:
                        for m in range(8, MT_QK):
                            emit_qkv_m(p + 1, m)
                    emit_proj(p)
                    del st[p]
    _split_sync_waits(nc)
    return nc


_NC_CACHE = {}


def _get_nc():
    if "nc" not in _NC_CACHE:
        _NC_CACHE["nc"] = build_nc()
    return _NC_CACHE["nc"]


def _prep(x, b_idx, w_qkv, q_bias, k_bias, v_bias, rel_pos_table, rel_index,
          w_proj, b_proj):
    x = np.asarray(x, dtype=np.float32)
    b_idx = np.asarray(b_idx)
    w_qkv = np.asarray(w_qkv, dtype=np.float32)
    q_bias = np.asarray(q_bias, dtype=np.float32)
    k_bias = np.asarray(k_bias, dtype=np.float32)
    v_bias = np.asarray(v_bias, dtype=np.float32)
    rel_pos_table = np.asarray(rel_pos_table, dtype=np.float32)
    rel_index = np.asarray(rel_index)
    w_proj = np.asarray(w_proj, dtype=np.float32)
    b_proj = np.asarray(b_proj, dtype=np.float32)

    scale = HD ** (-0.5)
    # fold attention scale into q weights/bias
    w_all = w_qkv.copy()
    w_all[0:C] *= scale
    wqkvT = np.ascontiguousarray(w_all.T)                      # [C, 3C]
    wqkv_p = wqkvT.reshape(KT, 128, 3 * C).transpose(1, 0, 2)  # [128, KT, 3C]
    wprojT = np.ascontiguousarray(w_proj.T)                    # [C, C]
    wproj_p = wprojT.reshape(KT, 128, C).transpose(1, 0, 2)    # [128, KT, C]

    # per-sample gathered biases
    qk_bias = np.concatenate([q_bias * scale, k_bias], axis=1)[b_idx]  # [B, 2C]
    qkvb_all = qk_bias.T.reshape(MT_QK, 128, B)                # [12, 128, B]
    projb_all = b_proj[b_idx].T.reshape(KT, 128, B)            # [6, 128, B]
    vb_all = v_bias[b_idx]                                     # [B, C]

    # exp of transposed relative-position bias: rpbT[m, h, n] = rpb[h][n, m]
    tbl = rel_pos_table[rel_index.reshape(-1)].reshape(N, N, H)  # [n, m, h]
    rpbT = np.exp(tbl.transpose(1, 2, 0))                        # [m, h, n]
    rpb_p = np.ascontiguousarray(rpbT, dtype=np.float32).astype(ml_dtypes.bfloat16)

    # x^T packed: [B, 128, KT*N] with partition p = c % 128, free (k, n)
    xT = x.transpose(0, 2, 1)                                  # [B, C, N]
    xt_p = xT.reshape(B, KT, 128, N).transpose(0, 2, 1, 3).reshape(B, 128, KT * N)
    xt_p = xt_p.astype(ml_dtypes.bfloat16)

    wqkv_p = np.ascontiguousarray(wqkv_p).astype(ml_dtypes.bfloat16)
    wproj_p = np.ascontiguousarray(wproj_p).astype(ml_dtypes.bfloat16)

    in_maps = []
    for i in range(NCORES):
        lo, hi = i * BL, (i + 1) * BL
        in_maps.append({
            "xt": np.ascontiguousarray(xt_p[lo:hi]),
            "wqkv": wqkv_p,
            "wproj": wproj_p,
            "rpb": rpb_p,
            "qkvb": np.ascontiguousarray(qkvb_all.transpose(1, 0, 2)[:, :, lo:hi]).astype(np.float32),
            "projb": np.ascontiguousarray(projb_all.transpose(1, 0, 2)[:, :, lo:hi]).astype(np.float32),
            "vb": np.ascontiguousarray(vb_all[lo:hi]).astype(ml_dtypes.bfloat16),
        })

    return in_maps


def _gather(results):
    outT = np.concatenate([results[i]["out"] for i in range(NCORES)], axis=0)
    return np.ascontiguousarray(outT.transpose(0, 2, 1))


def kernel(**inputs):
    in_maps = _prep(**inputs)
    nc = _get_nc()
    res = run_bass_kernel_spmd(nc, in_maps, list(range(NCORES))).results
    return _gather(res)



# revision 42
# speedup vs baseline: 3.5437x; 3.5437x over previous
"""BEiT-style attention (B=128, N=197, C=768, H=12) on 8 TRN2 NeuronCores.

Strategy: pure data parallelism over batch — each core processes 16
samples end-to-end; no collectives. Host pre-gathers the per-sample
bitfit biases (b_idx lookups), pre-transposes x to [C, N] per sample,
folds the attention scale into the q weights/bias, and pre-computes
exp(rel_pos_bias)^T so softmax(S + rpb) = normalize(exp(S) * exp_rpbT).

Device per sample:
  qkT  [1536,197] = w_qk @ x^T          (weights stationary, 2-sample batch)
  v    [197, 768] = x @ w_v^T + v_bias  (x^T stationary slices)
  per head: S^T[m,n] = k_h @ q_h^T  (two heads packed in the 128-row PE
  array via tile_position), P = exp(S^T) * exp_rpbT, out^T[hd+1, n] =
  [v_h | 1]^T @ P  (ones column yields the softmax denominator),
  normalize with a reciprocal broadcast via a ones-matmul,
  y^T [768,197] = w_proj @ out_allT + b_proj.
Host transposes the gathered y^T back to [B, N, C].
"""
import numpy as np
import ml_dtypes

import concourse.bass as bass
import concourse.tile as tile
from concourse import mybir
from concourse.bass_utils import run_bass_kernel_spmd

B, N, C = 128, 197, 768
H, HD = 12, 64
NCORES = 8
BL = B // NCORES          # 16 samples per core
NPAIRS = BL // 2          # 8 sample pairs
N2 = 2 * N                # 394
KT = C // 128             # 6 k-tiles
MT_QK = 2 * C // 128      # 12 m-tiles of qkT
BF16 = mybir.dt.bfloat16
F32 = mybir.dt.float32
F32R = mybir.dt.float32r
AF = mybir.ActivationFunctionType
PV_TRAIL = 1


def _split_sync_waits(nc, max_waits=1, max_updates=1):
    """TPB descriptors have ONE wait and ONE update slot; hoist extras
    onto same-engine NoOps (trailing-nop updates are completion-safe)."""
    n_split = 0
    for f in nc.m.functions:
        for bb in f.blocks:
            old = list(bb.instructions)
            new = []
            changed = False
            for ins in old:
                si = ins.sync_info
                tname = type(ins).__name__
                is_dma = ("TensorLoad" in tname or "TensorSave" in tname
                          or "TensorCopy" in tname or "TriggeredCopy" in tname)
                if si is None or is_dma:
                    new.append(ins)
                    continue
                waits = list(si.on_wait)
                updates = list(si.on_update)
                if len(waits) <= max_waits and len(updates) <= max_updates:
                    new.append(ins)
                    continue
                changed = True
                n_split += 1
                while len(waits) > max_waits:
                    w = waits.pop(0)
                    new.append(mybir.InstNoOp(
                        name=nc.get_next_instruction_name(), engine=ins.engine,
                        sync_info=mybir.SyncInfo(on_wait=[w], on_update=[]),
                        bass_nofuse=True))
                post = []
                while len(updates) > max_updates:
                    u = updates.pop()
                    post.append(mybir.InstNoOp(
                        name=nc.get_next_instruction_name(), engine=ins.engine,
                        sync_info=mybir.SyncInfo(on_wait=[], on_update=[u]),
                        bass_nofuse=True))
                ins.sync_info = mybir.SyncInfo(on_wait=waits, on_update=updates)
                new.append(ins)
                new.extend(post)
            if changed:
                bb.instructions = new
    return n_split


def build_nc(repeat=1):
    nc = bass.Bass("TRN2")
    xt_d = nc.declare_dram_parameter("xt", [BL, 128, KT * N], BF16, isOutput=False)
    wqk_d = nc.declare_dram_parameter("wqk", [128, MT_QK, KT, 128], BF16,
                                      isOutput=False)
    wv_d = nc.declare_dram_parameter("wv", [128, KT, C], BF16, isOutput=False)
    wproj_d = nc.declare_dram_parameter("wproj", [128, KT, C], BF16, isOutput=False)
    rpb_d = nc.declare_dram_parameter("rpb", [N, H, N], BF16, isOutput=False)
    qkvb_d = nc.declare_dram_parameter("qkvb", [128, MT_QK, BL], F32, isOutput=False)
    projb_d = nc.declare_dram_parameter("projb", [128, KT, BL], F32, isOutput=False)
    vb_d = nc.declare_dram_parameter("vb", [BL, H * HD], BF16, isOutput=False)
    out_d = nc.declare_dram_parameter("out", [BL, C, N], BF16, isOutput=True)

    with tile.TileContext(nc) as tc:
        with (
            tc.tile_pool(name="const", bufs=1) as const,
            tc.tile_pool(name="xtp", bufs=3) as xtp,
            tc.tile_pool(name="qkp", bufs=3) as qkp,
            tc.tile_pool(name="vp", bufs=4) as vp,
            tc.tile_pool(name="vbp", bufs=4) as vbp,
            tc.tile_pool(name="esp", bufs=8) as esp,
            tc.tile_pool(name="rcp", bufs=7) as rcp,
            tc.tile_pool(name="oap", bufs=3) as oap,
            tc.tile_pool(name="yp", bufs=3) as yp,
            tc.tile_pool(name="psA", bufs=2, space="PSUM") as psA,
            tc.tile_pool(name="psS", bufs=2, space="PSUM") as psS,
            tc.tile_pool(name="psB", bufs=2, space="PSUM") as psB,
        ):
            # ---- resident constants ----
            # Load order matters: the first qkT matmuls need only wqk m=0 +
            # xt(0) + qkvb. wqk is loaded per m-block (contiguous in DRAM)
            # so compute starts after ~200KB; everything else is emitted at
            # its last-moment point so its DMA traffic doesn't sit in front
            # of the critical path on the shared DMA engines.
            wqk_sb = const.tile([128, MT_QK, KT, 128], BF16)
            nc.sync.dma_start(wqk_sb[:, 0], wqk_d[:, 0])
            qkvb_sb = const.tile([128, MT_QK, BL], F32)
            nc.sync.dma_start(qkvb_sb, qkvb_d[:])
            wv_sb = const.tile([128, KT, C], BF16)
            rpb0 = const.tile([128, H, N], BF16)
            rpb1 = const.tile([69, H, N], BF16)
            wproj_sb = const.tile([128, KT, C], BF16)
            projb_sb = const.tile([128, KT, BL], F32)
            ones64b = const.tile([1, 64], BF16)
            nc.vector.memset(ones64b, 1.0)

            for _rep in range(repeat):
                st = {}

                def emit_load(p):
                    sg = (2 * p, 2 * p + 1)
                    xt = xtp.tile([128, 2, KT, N], BF16, name="xt")
                    for s in range(2):
                        nc.sync.dma_start(
                            xt[:, s].rearrange("p k n -> p (k n)"),
                            xt_d[sg[s]])
                    vb_bc = [None, None]
                    for s in range(2):
                        vb_bc[s] = vbp.tile([128, H, HD], BF16, tag="vb", name="vb")
                        nc.scalar.dma_start(
                            vb_bc[s],
                            vb_d[sg[s]:sg[s] + 1, :].rearrange(
                                "o (h d) -> o h d", h=H).to_broadcast([128, H, HD]))
                    qkT = qkp.tile([128, MT_QK, N2], BF16, name="qkT")
                    st[p] = {"xt": xt, "vb": vb_bc, "qkT": qkT, "v": None}

                def emit_qkv_m(p, m):
                    sg = (2 * p, 2 * p + 1)
                    xt, qkT = st[p]["xt"], st[p]["qkT"]
                    ps = psA.tile([128, N2], F32, tag="mm", name="ps")
                    for k in range(KT):
                        nc.tensor.matmul(ps, wqk_sb[:, m, k, :],
                                         xt[:, :, k, :], start=(k == 0),
                                         stop=(k == KT - 1))
                    for s in range(2):
                        dst = qkT[:, m, s * N:(s + 1) * N]
                        src = ps[:, s * N:(s + 1) * N]
                        bias = qkvb_sb[:, m, sg[s]:sg[s] + 1]
                        if m % 2 == 0:
                            nc.scalar.activation(dst, src, AF.Identity,
                                                 bias=bias, scale=1.0)
                        else:
                            nc.vector.tensor_scalar_add(dst, src, bias)

                def emit_v_chunk(p, s, nt):
                    xt, vb_bc = st[p]["xt"], st[p]["vb"]
                    if st[p]["v"] is None:
                        st[p]["v"] = [[None, None], [None, None]]
                    nts = 128 if nt == 0 else N - 128
                    vt = vp.tile([nts, H, HD + 1], BF16, tag=f"v{nt}", name="vt")
                    nc.vector.memset(vt[:, :, HD:HD + 1], 1.0)
                    for half in range(2):
                        ps = psA.tile([128, N2], F32, tag="mm", name="ps")
                        for k in range(KT):
                            nc.tensor.matmul(
                                ps[:nts, 0:384],
                                xt[:, s, k, nt * 128:nt * 128 + nts],
                                wv_sb[:, k, half * 384:(half + 1) * 384],
                                start=(k == 0), stop=(k == KT - 1))
                        # fused psum drain + bitfit v-bias add, all on DVE so
                        # vt has a single producer engine (fewer sem waits)
                        nc.vector.scalar_tensor_tensor(
                            vt[:, half * 6:(half + 1) * 6, 0:HD],
                            ps[:nts, 0:384].rearrange("p (h d) -> p h d", h=6),
                            0.0,
                            vb_bc[s][:nts, half * 6:(half + 1) * 6, :],
                            mybir.AluOpType.bypass,
                            mybir.AluOpType.add)
                    st[p]["v"][s][nt] = vt

                def emit_v(p):
                    for s in range(2):
                        for nt in range(2):
                            emit_v_chunk(p, s, nt)

                def emit_s(p, s, hp):
                    # The two concurrent row-packed matmuls drain into the
                    # two DIFFERENT banks of one 2-bank psum tile (same-bank
                    # concurrent drain is a HW error), so exp and the rpb
                    # multiply each run as ONE wide instruction.
                    qkT = st[p]["qkT"]
                    ha = 2 * hp
                    es = [None, None]
                    for mt in range(2):
                        mts = 128 if mt == 0 else N - 128
                        rpb = rpb0 if mt == 0 else rpb1
                        pss = psS.tile([128, 2, 512], F32, tag="s2", name="pss")
                        for hh, (pl, pr) in enumerate(((0, 64), (64, 128))):
                            nc.tensor.matmul(
                                pss[:mts, hh, 0:N],
                                qkT[pl:pr, KT + hp,
                                    s * N + mt * 128: s * N + mt * 128 + mts],
                                qkT[pl:pr, hp, s * N:(s + 1) * N],
                                start=True, stop=True, tile_position=(pl, 0))
                        e = esp.tile([mts, N2], BF16, tag=f"es{mt}", name="es")
                        nc.scalar.activation(
                            e.rearrange("p (a n) -> p a n", a=2),
                            pss[:mts, :, 0:N], AF.Exp)
                        # SBUF-only multiply: mt=0 rides the otherwise-idle
                        # Pool engine, mt=1 stays on DVE
                        eng = nc.gpsimd if mt == 0 else nc.vector
                        eng.tensor_mul(
                            e, e,
                            rpb[:, ha:ha + 2, :].rearrange("p a n -> p (a n)"))
                        es[mt] = e
                    return es

                def emit_pv(p, s, hp, es, oa):
                    v_sb = st[p]["v"]
                    pvt = psB.tile([HD + 1, N2], F32, tag="pv", name="pvt")
                    for mt in range(2):
                        for hh, h in enumerate((2 * hp, 2 * hp + 1)):
                            # start=True clears has_written for the WHOLE bank:
                            # only the very first matmul may set it.
                            nc.tensor.matmul(
                                pvt[:, hh * N:(hh + 1) * N],
                                v_sb[s][mt][:, h, :],
                                es[mt][:, hh * N:(hh + 1) * N],
                                start=(mt == 0 and hh == 0),
                                stop=(mt == 1 and hh == 1))
                    # division-free denominator: 1/d = exp(-ln d) on Act
                    # (Ln+Exp share one act table), broadcast via PE, plain
                    # DVE copy to SBUF (no iterative-divide op anywhere)
                    t = rcp.tile([1, N2], F32, tag="rc", name="t")
                    nc.scalar.activation(t, pvt[HD:HD + 1, :], AF.Ln)
                    r = rcp.tile([1, N2], BF16, tag="rc2", name="r")
                    nc.scalar.activation(r, t, AF.Exp, scale=-1.0)
                    bc = psB.tile([64, N2], F32, tag="pv", name="bc")
                    nc.tensor.matmul(bc, ones64b, r, start=True, stop=True)
                    bc_sb = rcp.tile([64, N2], BF16, tag="bc_sb", name="bc_sb")
                    with nc.allow_low_precision(reason="softmax denom in bf16"):
                        nc.vector.tensor_copy(bc_sb, bc)
                    for hh in range(2):
                        nc.vector.tensor_mul(
                            oa[hh * 64:(hh + 1) * 64, hp, s * N:(s + 1) * N],
                            pvt[0:HD, hh * N:(hh + 1) * N],
                            bc_sb[:, hh * N:(hh + 1) * N])

                def emit_proj_m(p, m):
                    sg = (2 * p, 2 * p + 1)
                    oa = st[p]["oa"]
                    ps = psA.tile([128, N2], F32, tag="mm", name="ps")
                    for k in range(KT):
                        nc.tensor.matmul(
                            ps, wproj_sb[:, k, m * 128:(m + 1) * 128],
                            oa[:, k, :], start=(k == 0), stop=(k == KT - 1))
                    y = yp.tile([128, N2], BF16, tag="y", name="y")
                    for s in range(2):
                        dst = y[:, s * N:(s + 1) * N]
                        src = ps[:, s * N:(s + 1) * N]
                        bias = projb_sb[:, m, sg[s]:sg[s] + 1]
                        if m % 2 == 0:
                            nc.scalar.activation(dst, src, AF.Identity,
                                                 bias=bias, scale=1.0)
                        else:
                            nc.vector.tensor_scalar_add(dst, src, bias)
                    for s in range(2):
                        nc.sync.dma_start(
                            out_d[sg[s], m * 128:(m + 1) * 128, :],
                            y[:, s * N:(s + 1) * N])

                # skewed pipeline: pair p attention carries pair p+1 QKV
                emit_load(0)
                if _rep == 0:
                    for m in range(1, MT_QK):
                        nc.sync.dma_start(wqk_sb[:, m], wqk_d[:, m])
                for m in range(MT_QK):
                    emit_qkv_m(0, m)
                if _rep == 0:
                    # tiny WAW guards: the scheduler would otherwise hoist
                    # these dependency-free loads in front of xt(0)/wqk on
                    # the shared DMA engines
                    qk0 = st[0]["qkT"]
                    nc.vector.tensor_copy(rpb0[0:1, 0, 0:1], qk0[0:1, 0, 0:1])
                    nc.vector.tensor_copy(rpb1[0:1, 0, 0:1], qk0[0:1, 0, 0:1])
                    nc.sync.dma_start(wv_sb, wv_d[:])
                    nc.scalar.dma_start(rpb0, rpb_d[0:128])
                    nc.scalar.dma_start(rpb1, rpb_d[128:N])
                emit_v(0)
                # proj of pair p-1 is spread through pair p's S-loop so the
                # softmax-denominator chain of p-1's last head-pairs overlaps
                # with p's matmuls instead of stalling the PE at the boundary
                iters = [(s, hp) for s in range(2) for hp in range(H // 2)]
                for p in range(NPAIRS):
                    if p + 1 < NPAIRS:
                        emit_load(p + 1)
                    oa = oap.tile([128, KT, N2], BF16, name="oa")
                    st[p]["oa"] = oa
                    pending = []
                    for i, (s, hp) in enumerate(iters):
                        if _rep == 0 and p == 0 and i == 6:
                            nc.vector.tensor_copy(wproj_sb[0:1, 0, 0:1],
                                                  st[0]["qkT"][0:1, 0, 0:1])
                            nc.sync.dma_start(projb_sb, projb_d[:])
                            nc.scalar.dma_start(wproj_sb, wproj_d[:])
                        es = emit_s(p, s, hp)
                        if p > 0:
                            if p == NPAIRS - 1:
                                # last pair: no next-pair qkv to interleave,
                                # so spread the previous proj over all iters
                                if i % 2 == 1:
                                    emit_proj_m(p - 1, (i - 1) // 2)
                            elif 1 <= i <= KT:
                                emit_proj_m(p - 1, i - 1)
                        if p + 1 < NPAIRS:
                            if i < 8:
                                emit_qkv_m(p + 1, i)
                            else:
                                emit_v_chunk(p + 1, (i - 8) // 2, (i - 8) % 2)
                        pending.append((s, hp, es))
                        if len(pending) > PV_TRAIL:
                            emit_pv(p, *pending.pop(0), oa)
                    for item in pending:
                        emit_pv(p, *item, oa)
                    if p + 1 < NPAIRS:
                        for m in range(8, MT_QK):
                            emit_qkv_m(p + 1, m)
                    if p > 0:
                        del st[p - 1]
                for m in range(KT):
                    emit_proj_m(NPAIRS - 1, m)
                del st[NPAIRS - 1]
    _split_sync_waits(nc)
    return nc


_NC_CACHE = {}


def _get_nc():
    if "nc" not in _NC_CACHE:
        _NC_CACHE["nc"] = build_nc()
    return _NC_CACHE["nc"]


def _prep(x, b_idx, w_qkv, q_bias, k_bias, v_bias, rel_pos_table, rel_index,
          w_proj, b_proj):
    x = np.asarray(x, dtype=np.float32)
    b_idx = np.asarray(b_idx)
    w_qkv = np.asarray(w_qkv, dtype=np.float32)
    q_bias = np.asarray(q_bias, dtype=np.float32)
    k_bias = np.asarray(k_bias, dtype=np.float32)
    v_bias = np.asarray(v_bias, dtype=np.float32)
    rel_pos_table = np.asarray(rel_pos_table, dtype=np.float32)
    rel_index = np.asarray(rel_index)
    w_proj = np.asarray(w_proj, dtype=np.float32)
    b_proj = np.asarray(b_proj, dtype=np.float32)

    scale = HD ** (-0.5)
    # fold attention scale into q weights/bias
    w_all = w_qkv.copy()
    w_all[0:C] *= scale
    wqkvT = np.ascontiguousarray(w_all.T)                      # [C, 3C]
    wqkv_p = wqkvT.reshape(KT, 128, 3 * C).transpose(1, 0, 2)  # [128, KT, 3C]
    wprojT = np.ascontiguousarray(w_proj.T)                    # [C, C]
    wproj_p = wprojT.reshape(KT, 128, C).transpose(1, 0, 2)    # [128, KT, C]

    # per-sample gathered biases
    qk_bias = np.concatenate([q_bias * scale, k_bias], axis=1)[b_idx]  # [B, 2C]
    qkvb_all = qk_bias.T.reshape(MT_QK, 128, B)                # [12, 128, B]
    projb_all = b_proj[b_idx].T.reshape(KT, 128, B)            # [6, 128, B]
    vb_all = v_bias[b_idx]                                     # [B, C]

    # exp of transposed relative-position bias: rpbT[m, h, n] = rpb[h][n, m]
    tbl = rel_pos_table[rel_index.reshape(-1)].reshape(N, N, H)  # [n, m, h]
    rpbT = np.exp(tbl.transpose(1, 2, 0))                        # [m, h, n]
    rpb_p = np.ascontiguousarray(rpbT, dtype=np.float32).astype(ml_dtypes.bfloat16)

    # x^T packed: [B, 128, KT*N] with partition p = c % 128, free (k, n)
    xT = x.transpose(0, 2, 1)                                  # [B, C, N]
    xt_p = xT.reshape(B, KT, 128, N).transpose(0, 2, 1, 3).reshape(B, 128, KT * N)
    xt_p = xt_p.astype(ml_dtypes.bfloat16)

    wqkv_p = np.ascontiguousarray(wqkv_p).astype(ml_dtypes.bfloat16)
    # wqk in m-block-major layout [128, MT_QK, KT, 128] so each m-block is
    # one contiguous DMA
    wqk_p = np.ascontiguousarray(
        wqkv_p[:, :, 0:2 * C].reshape(128, KT, MT_QK, 128).transpose(0, 2, 1, 3))
    wv_p = np.ascontiguousarray(wqkv_p[:, :, 2 * C:3 * C])
    wproj_p = np.ascontiguousarray(wproj_p).astype(ml_dtypes.bfloat16)

    in_maps = []
    for i in range(NCORES):
        lo, hi = i * BL, (i + 1) * BL
        in_maps.append({
            "xt": np.ascontiguousarray(xt_p[lo:hi]),
            "wqk": wqk_p,
            "wv": wv_p,
            "wproj": wproj_p,
            "rpb": rpb_p,
            "qkvb": np.ascontiguousarray(qkvb_all.transpose(1, 0, 2)[:, :, lo:hi]).astype(np.float32),
            "projb": np.ascontiguousarray(projb_all.transpose(1, 0, 2)[:, :, lo:hi]).astype(np.float32),
            "vb": np.ascontiguousarray(vb_all[lo:hi]).astype(ml_dtypes.bfloat16),
        })

    return in_maps


def _gather(results):
    outT = np.concatenate([results[i]["out"] for i in range(NCORES)], axis=0)
    return np.ascontiguousarray(outT.astype(np.float32).transpose(0, 2, 1))


def kernel(**inputs):
    in_maps = _prep(**inputs)
    nc = _get_nc()
    res = run_bass_kernel_spmd(nc, in_maps, list(range(NCORES))).results
    return _gather(res)



# revision 43
# speedup vs baseline: 4.3970x; 1.2408x over previous
"""BEiT-style attention (B=128, N=197, C=768, H=12) on 8 TRN2 NeuronCores.

Strategy: pure data parallelism over batch — each core processes 16
samples end-to-end; no collectives. Host pre-gathers the per-sample
bitfit biases (b_idx lookups), pre-transposes x to [C, N] per sample,
folds the attention scale into the q weights/bias, and pre-computes
exp(rel_pos_bias)^T so softmax(S + rpb) = normalize(exp(S) * exp_rpbT).

Device per sample:
  qkT  [1536,197] = w_qk @ x^T          (weights stationary, 2-sample batch)
  v    [197, 768] = x @ w_v^T + v_bias  (x^T stationary slices)
  per head: S^T[m,n] = k_h @ q_h^T  (two heads packed in the 128-row PE
  array via tile_position), P = exp(S^T) * exp_rpbT, out^T[hd+1, n] =
  [v_h | 1]^T @ P  (ones column yields the softmax denominator),
  normalize with a reciprocal broadcast via a ones-matmul,
  y^T [768,197] = w_proj @ out_allT + b_proj.
Host transposes the gathered y^T back to [B, N, C].
"""
import numpy as np
import ml_dtypes

import concourse.bass as bass
import concourse.tile as tile
from concourse import mybir
from concourse.bass_utils import run_bass_kernel_spmd

B, N, C = 128, 197, 768
H, HD = 12, 64
NCORES = 8
BL = B // NCORES          # 16 samples per core
NPAIRS = BL // 2          # 8 sample pairs
N2 = 2 * N                # 394
KT = C // 128             # 6 k-tiles
MT_QK = 2 * C // 128      # 12 m-tiles of qkT
BF16 = mybir.dt.bfloat16
F32 = mybir.dt.float32
F32R = mybir.dt.float32r
AF = mybir.ActivationFunctionType
PV_TRAIL = 1


def _split_sync_waits(nc, max_waits=1, max_updates=1):
    """TPB descriptors have ONE wait and ONE update slot; hoist extras
    onto same-engine NoOps (trailing-nop updates are completion-safe)."""
    n_split = 0
    for f in nc.m.functions:
        for bb in f.blocks:
            old = list(bb.instructions)
            new = []
            changed = False
            for ins in old:
                si = ins.sync_info
                tname = type(ins).__name__
                is_dma = ("TensorLoad" in tname or "TensorSave" in tname
                          or "TensorCopy" in tname or "TriggeredCopy" in tname)
                if si is None or is_dma:
                    new.append(ins)
                    continue
                waits = list(si.on_wait)
                updates = list(si.on_update)
                if len(waits) <= max_waits and len(updates) <= max_updates:
                    new.append(ins)
                    continue
                changed = True
                n_split += 1
                while len(waits) > max_waits:
                    w = waits.pop(0)
                    new.append(mybir.InstNoOp(
                        name=nc.get_next_instruction_name(), engine=ins.engine,
                        sync_info=mybir.SyncInfo(on_wait=[w], on_update=[]),
                        bass_nofuse=True))
                post = []
                while len(updates) > max_updates:
                    u = updates.pop()
                    post.append(mybir.InstNoOp(
                        name=nc.get_next_instruction_name(), engine=ins.engine,
                        sync_info=mybir.SyncInfo(on_wait=[], on_update=[u]),
                        bass_nofuse=True))
                ins.sync_info = mybir.SyncInfo(on_wait=waits, on_update=updates)
                new.append(ins)
                new.extend(post)
            if changed:
                bb.instructions = new
    return n_split


def build_nc(repeat=1):
    nc = bass.Bass("TRN2")
    xt_d = nc.declare_dram_parameter("xt", [BL, 128, KT * N], BF16, isOutput=False)
    wqk_d = nc.declare_dram_parameter("wqk", [128, MT_QK, KT, 128], BF16,
                                      isOutput=False)
    wv_d = nc.declare_dram_parameter("wv", [128, KT, C], BF16, isOutput=False)
    wproj_d = nc.declare_dram_parameter("wproj", [128, KT, C], BF16, isOutput=False)
    rpb_d = nc.declare_dram_parameter("rpb", [N, H, N], BF16, isOutput=False)
    qkvb_d = nc.declare_dram_parameter("qkvb", [128, MT_QK, BL], F32, isOutput=False)
    projb_d = nc.declare_dram_parameter("projb", [128, KT, BL], F32, isOutput=False)
    vb_d = nc.declare_dram_parameter("vb", [BL, H * HD], BF16, isOutput=False)
    out_d = nc.declare_dram_parameter("out", [BL, C, N], BF16, isOutput=True)

    with tile.TileContext(nc) as tc:
        with (
            tc.tile_pool(name="const", bufs=1) as const,
            tc.tile_pool(name="xtp", bufs=3) as xtp,
            tc.tile_pool(name="qkp", bufs=3) as qkp,
            tc.tile_pool(name="vp", bufs=4) as vp,
            tc.tile_pool(name="vbp", bufs=4) as vbp,
            tc.tile_pool(name="esp", bufs=8) as esp,
            tc.tile_pool(name="rcp", bufs=7) as rcp,
            tc.tile_pool(name="oap", bufs=3) as oap,
            tc.tile_pool(name="yp", bufs=3) as yp,
            tc.tile_pool(name="psA", bufs=2, space="PSUM") as psA,
            tc.tile_pool(name="psS", bufs=2, space="PSUM") as psS,
            tc.tile_pool(name="psB", bufs=2, space="PSUM") as psB,
        ):
            # ---- resident constants ----
            # Load order matters: the first qkT matmuls need only wqk m=0 +
            # xt(0) + qkvb. wqk is loaded per m-block (contiguous in DRAM)
            # so compute starts after ~200KB; everything else is emitted at
            # its last-moment point so its DMA traffic doesn't sit in front
            # of the critical path on the shared DMA engines.
            wqk_sb = const.tile([128, MT_QK, KT, 128], BF16)
            nc.sync.dma_start(wqk_sb[:, 0], wqk_d[:, 0])
            qkvb_sb = const.tile([128, MT_QK, BL], F32)
            nc.sync.dma_start(qkvb_sb, qkvb_d[:])
            wv_sb = const.tile([128, KT, C], BF16)
            rpb0 = const.tile([128, H, N], BF16)
            rpb1 = const.tile([69, H, N], BF16)
            wproj_sb = const.tile([128, KT, C], BF16)
            projb_sb = const.tile([128, KT, BL], F32)
            ones64b = const.tile([1, 64], BF16)
            nc.vector.memset(ones64b, 1.0)

            for _rep in range(repeat):
                st = {}

                def emit_load(p):
                    sg = (2 * p, 2 * p + 1)
                    xt = xtp.tile([128, 2, KT, N], BF16, name="xt")
                    for s in range(2):
                        nc.sync.dma_start(
                            xt[:, s].rearrange("p k n -> p (k n)"),
                            xt_d[sg[s]])
                    vb_bc = [None, None]
                    for s in range(2):
                        vb_bc[s] = vbp.tile([128, H, HD], BF16, tag="vb", name="vb")
                        nc.scalar.dma_start(
                            vb_bc[s],
                            vb_d[sg[s]:sg[s] + 1, :].rearrange(
                                "o (h d) -> o h d", h=H).to_broadcast([128, H, HD]))
                    qkT = qkp.tile([128, MT_QK, N2], BF16, name="qkT")
                    st[p] = {"xt": xt, "vb": vb_bc, "qkT": qkT, "v": None}

                def emit_qkv_m(p, m):
                    sg = (2 * p, 2 * p + 1)
                    xt, qkT = st[p]["xt"], st[p]["qkT"]
                    ps = psA.tile([128, N2], F32, tag="mm", name="ps")
                    for k in range(KT):
                        nc.tensor.matmul(ps, wqk_sb[:, m, k, :],
                                         xt[:, :, k, :], start=(k == 0),
                                         stop=(k == KT - 1))
                    for s in range(2):
                        dst = qkT[:, m, s * N:(s + 1) * N]
                        src = ps[:, s * N:(s + 1) * N]
                        bias = qkvb_sb[:, m, sg[s]:sg[s] + 1]
                        if m % 2 == 0:
                            nc.scalar.activation(dst, src, AF.Identity,
                                                 bias=bias, scale=1.0)
                        else:
                            nc.vector.tensor_scalar_add(dst, src, bias)

                def emit_v_chunk(p, s, nt):
                    xt, vb_bc = st[p]["xt"], st[p]["vb"]
                    if st[p]["v"] is None:
                        st[p]["v"] = [[None, None], [None, None]]
                    nts = 128 if nt == 0 else N - 128
                    vt = vp.tile([nts, H, HD + 1], BF16, tag=f"v{nt}", name="vt")
                    nc.vector.memset(vt[:, :, HD:HD + 1], 1.0)
                    for half in range(2):
                        ps = psA.tile([128, N2], F32, tag="mm", name="ps")
                        for k in range(KT):
                            nc.tensor.matmul(
                                ps[:nts, 0:384],
                                xt[:, s, k, nt * 128:nt * 128 + nts],
                                wv_sb[:, k, half * 384:(half + 1) * 384],
                                start=(k == 0), stop=(k == KT - 1))
                        # fused psum drain + bitfit v-bias add, all on DVE so
                        # vt has a single producer engine (fewer sem waits)
                        nc.vector.scalar_tensor_tensor(
                            vt[:, half * 6:(half + 1) * 6, 0:HD],
                            ps[:nts, 0:384].rearrange("p (h d) -> p h d", h=6),
                            0.0,
                            vb_bc[s][:nts, half * 6:(half + 1) * 6, :],
                            mybir.AluOpType.bypass,
                            mybir.AluOpType.add)
                    st[p]["v"][s][nt] = vt

                def emit_v(p):
                    for s in range(2):
                        for nt in range(2):
                            emit_v_chunk(p, s, nt)

                def emit_s(p, s, hp):
                    # The two concurrent row-packed matmuls drain into the
                    # two DIFFERENT banks of one 2-bank psum tile (same-bank
                    # concurrent drain is a HW error), so exp and the rpb
                    # multiply each run as ONE wide instruction.
                    qkT = st[p]["qkT"]
                    ha = 2 * hp
                    es = [None, None]
                    for mt in range(2):
                        mts = 128 if mt == 0 else N - 128
                        rpb = rpb0 if mt == 0 else rpb1
                        pss = psS.tile([128, 2, 512], F32, tag="s2", name="pss")
                        for hh, (pl, pr) in enumerate(((0, 64), (64, 128))):
                            nc.tensor.matmul(
                                pss[:mts, hh, 0:N],
                                qkT[pl:pr, KT + hp,
                                    s * N + mt * 128: s * N + mt * 128 + mts],
                                qkT[pl:pr, hp, s * N:(s + 1) * N],
                                start=True, stop=True, tile_position=(pl, 0))
                        e = esp.tile([mts, N2], BF16, tag=f"es{mt}", name="es")
                        nc.scalar.activation(
                            e.rearrange("p (a n) -> p a n", a=2),
                            pss[:mts, :, 0:N], AF.Exp)
                        # SBUF-only multiply: mt=0 rides the otherwise-idle
                        # Pool engine, mt=1 stays on DVE
                        eng = nc.gpsimd if mt == 0 else nc.vector
                        eng.tensor_mul(
                            e, e,
                            rpb[:, ha:ha + 2, :].rearrange("p a n -> p (a n)"))
                        es[mt] = e
                    return es

                def emit_pv(p, s, hp, es, oa):
                    v_sb = st[p]["v"]
                    pvt = psB.tile([HD + 1, N2], F32, tag="pv", name="pvt")
                    for mt in range(2):
                        for hh, h in enumerate((2 * hp, 2 * hp + 1)):
                            # start=True clears has_written for the WHOLE bank:
                            # only the very first matmul may set it.
                            nc.tensor.matmul(
                                pvt[:, hh * N:(hh + 1) * N],
                                v_sb[s][mt][:, h, :],
                                es[mt][:, hh * N:(hh + 1) * N],
                                start=(mt == 0 and hh == 0),
                                stop=(mt == 1 and hh == 1))
                    # denominator: Act drains the PSUM row, PE broadcasts it
                    # to 64 partitions, DVE takes a fast approximate
                    # reciprocal into SBUF for the normalize muls
                    ssum = rcp.tile([1, N2], BF16, tag="rc", name="ssum")
                    nc.scalar.copy(ssum, pvt[HD:HD + 1, :])
                    bc = psB.tile([64, N2], F32, tag="pv", name="bc")
                    nc.tensor.matmul(bc, ones64b, ssum, start=True, stop=True)
                    bc_sb = rcp.tile([64, N2], BF16, tag="bc_sb", name="bc_sb")
                    with nc.allow_low_precision(reason="softmax denom in bf16"):
                        nc.vector.reciprocal(bc_sb, bc)
                    for hh in range(2):
                        nc.vector.tensor_mul(
                            oa[hh * 64:(hh + 1) * 64, hp, s * N:(s + 1) * N],
                            pvt[0:HD, hh * N:(hh + 1) * N],
                            bc_sb[:, hh * N:(hh + 1) * N])

                def emit_proj_m(p, m):
                    sg = (2 * p, 2 * p + 1)
                    oa = st[p]["oa"]
                    ps = psA.tile([128, N2], F32, tag="mm", name="ps")
                    for k in range(KT):
                        nc.tensor.matmul(
                            ps, wproj_sb[:, k, m * 128:(m + 1) * 128],
                            oa[:, k, :], start=(k == 0), stop=(k == KT - 1))
                    y = yp.tile([128, N2], BF16, tag="y", name="y")
                    for s in range(2):
                        dst = y[:, s * N:(s + 1) * N]
                        src = ps[:, s * N:(s + 1) * N]
                        bias = projb_sb[:, m, sg[s]:sg[s] + 1]
                        if m % 2 == 0:
                            nc.scalar.activation(dst, src, AF.Identity,
                                                 bias=bias, scale=1.0)
                        else:
                            nc.vector.tensor_scalar_add(dst, src, bias)
                    for s in range(2):
                        nc.sync.dma_start(
                            out_d[sg[s], m * 128:(m + 1) * 128, :],
                            y[:, s * N:(s + 1) * N])

                # skewed pipeline: pair p attention carries pair p+1 QKV
                emit_load(0)
                if _rep == 0:
                    for m in range(1, MT_QK):
                        nc.sync.dma_start(wqk_sb[:, m], wqk_d[:, m])
                for m in range(MT_QK):
                    emit_qkv_m(0, m)
                if _rep == 0:
                    # tiny WAW guards: the scheduler would otherwise hoist
                    # these dependency-free loads in front of xt(0)/wqk on
                    # the shared DMA engines
                    qk0 = st[0]["qkT"]
                    nc.vector.tensor_copy(rpb0[0:1, 0, 0:1], qk0[0:1, 0, 0:1])
                    nc.vector.tensor_copy(rpb1[0:1, 0, 0:1], qk0[0:1, 0, 0:1])
                    nc.sync.dma_start(wv_sb, wv_d[:])
                    nc.scalar.dma_start(rpb0, rpb_d[0:128])
                    nc.scalar.dma_start(rpb1, rpb_d[128:N])
                emit_v(0)
                # proj of pair p-1 is spread through pair p's S-loop so the
                # softmax-denominator chain of p-1's last head-pairs overlaps
                # with p's matmuls instead of stalling the PE at the boundary
                iters = [(s, hp) for s in range(2) for hp in range(H // 2)]
                for p in range(NPAIRS):
                    if p + 1 < NPAIRS:
                        emit_load(p + 1)
                    oa = oap.tile([128, KT, N2], BF16, name="oa")
                    st[p]["oa"] = oa
                    pending = []
                    for i, (s, hp) in enumerate(iters):
                        if _rep == 0 and p == 0 and i == 6:
                            nc.vector.tensor_copy(wproj_sb[0:1, 0, 0:1],
                                                  st[0]["qkT"][0:1, 0, 0:1])
                            nc.sync.dma_start(projb_sb, projb_d[:])
                            nc.scalar.dma_start(wproj_sb, wproj_d[:])
                        es = emit_s(p, s, hp)
                        if p > 0:
                            if p == NPAIRS - 1:
                                # last pair: no next-pair qkv to interleave,
                                # so spread the previous proj over all iters
                                if i % 2 == 1:
                                    emit_proj_m(p - 1, (i - 1) // 2)
                            elif 1 <= i <= KT:
                                emit_proj_m(p - 1, i - 1)
                        if p + 1 < NPAIRS:
                            if i < 8:
                                emit_qkv_m(p + 1, i)
                            else:
                                emit_v_chunk(p + 1, (i - 8) // 2, (i - 8) % 2)
                        pending.append((s, hp, es))
                        if len(pending) > PV_TRAIL:
                            emit_pv(p, *pending.pop(0), oa)
                    for item in pending:
                        emit_pv(p, *item, oa)
                    if p + 1 < NPAIRS:
                        for m in range(8, MT_QK):
                            emit_qkv_m(p + 1, m)
                    if p > 0:
                        del st[p - 1]
                for m in range(KT):
                    emit_proj_m(NPAIRS - 1, m)
                del st[NPAIRS - 1]
    _split_sync_waits(nc)
    return nc


_NC_CACHE = {}


def _get_nc():
    if "nc" not in _NC_CACHE:
        _NC_CACHE["nc"] = build_nc()
    return _NC_CACHE["nc"]


def _prep(x, b_idx, w_qkv, q_bias, k_bias, v_bias, rel_pos_table, rel_index,
          w_proj, b_proj):
    x = np.asarray(x, dtype=np.float32)
    b_idx = np.asarray(b_idx)
    w_qkv = np.asarray(w_qkv, dtype=np.float32)
    q_bias = np.asarray(q_bias, dtype=np.float32)
    k_bias = np.asarray(k_bias, dtype=np.float32)
    v_bias = np.asarray(v_bias, dtype=np.float32)
    rel_pos_table = np.asarray(rel_pos_table, dtype=np.float32)
    rel_index = np.asarray(rel_index)
    w_proj = np.asarray(w_proj, dtype=np.float32)
    b_proj = np.asarray(b_proj, dtype=np.float32)

    scale = HD ** (-0.5)
    # fold attention scale into q weights/bias
    w_all = w_qkv.copy()
    w_all[0:C] *= scale
    wqkvT = np.ascontiguousarray(w_all.T)                      # [C, 3C]
    wqkv_p = wqkvT.reshape(KT, 128, 3 * C).transpose(1, 0, 2)  # [128, KT, 3C]
    wprojT = np.ascontiguousarray(w_proj.T)                    # [C, C]
    wproj_p = wprojT.reshape(KT, 128, C).transpose(1, 0, 2)    # [128, KT, C]

    # per-sample gathered biases
    qk_bias = np.concatenate([q_bias * scale, k_bias], axis=1)[b_idx]  # [B, 2C]
    qkvb_all = qk_bias.T.reshape(MT_QK, 128, B)                # [12, 128, B]
    projb_all = b_proj[b_idx].T.reshape(KT, 128, B)            # [6, 128, B]
    vb_all = v_bias[b_idx]                                     # [B, C]

    # exp of transposed relative-position bias: rpbT[m, h, n] = rpb[h][n, m]
    tbl = rel_pos_table[rel_index.reshape(-1)].reshape(N, N, H)  # [n, m, h]
    rpbT = np.exp(tbl.transpose(1, 2, 0))                        # [m, h, n]
    rpb_p = np.ascontiguousarray(rpbT, dtype=np.float32).astype(ml_dtypes.bfloat16)

    # x^T packed: [B, 128, KT*N] with partition p = c % 128, free (k, n)
    xT = x.transpose(0, 2, 1)                                  # [B, C, N]
    xt_p = xT.reshape(B, KT, 128, N).transpose(0, 2, 1, 3).reshape(B, 128, KT * N)
    xt_p = xt_p.astype(ml_dtypes.bfloat16)

    wqkv_p = np.ascontiguousarray(wqkv_p).astype(ml_dtypes.bfloat16)
    # wqk in m-block-major layout [128, MT_QK, KT, 128] so each m-block is
    # one contiguous DMA
    wqk_p = np.ascontiguousarray(
        wqkv_p[:, :, 0:2 * C].reshape(128, KT, MT_QK, 128).transpose(0, 2, 1, 3))
    wv_p = np.ascontiguousarray(wqkv_p[:, :, 2 * C:3 * C])
    wproj_p = np.ascontiguousarray(wproj_p).astype(ml_dtypes.bfloat16)

    in_maps = []
    for i in range(NCORES):
        lo, hi = i * BL, (i + 1) * BL
        in_maps.append({
            "xt": np.ascontiguousarray(xt_p[lo:hi]),
            "wqk": wqk_p,
            "wv": wv_p,
            "wproj": wproj_p,
            "rpb": rpb_p,
            "qkvb": np.ascontiguousarray(qkvb_all.transpose(1, 0, 2)[:, :, lo:hi]).astype(np.float32),
            "projb": np.ascontiguousarray(projb_all.transpose(1, 0, 2)[:, :, lo:hi]).astype(np.float32),
            "vb": np.ascontiguousarray(vb_all[lo:hi]).astype(ml_dtypes.bfloat16),
        })

    return in_maps


def _gather(results):
    outT = np.concatenate([results[i]["out"] for i in range(NCORES)], axis=0)
    return np.ascontiguousarray(outT.astype(np.float32).transpose(0, 2, 1))


def kernel(**inputs):
    in_maps = _prep(**inputs)
    nc = _get_nc()
    res = run_bass_kernel_spmd(nc, in_maps, list(range(NCORES))).results
    return _gather(res)

